# revision 23
# baseline (speedup 1.0000x reference)
"""Trainium2 Bass kernel for nn_ANFISRuleStrengthLayer.

Math (N_INPUTS=2, N_MF=64): out[b, j1*64 + j2] = x[b, 0, j1] * x[b, 1, j2]
i.e. a per-sample outer product of two 64-vectors.
Input  x:   (32768, 2, 64) f32
Output out: (32768, 4096)  f32   (512 MiB -> heavily DMA-write bound)

Sharding: pure data parallel over batch across 8 NeuronCores
(4096 rows per core, no cross-core communication).
"""

import os
from contextlib import ExitStack

import numpy as np

import concourse.bass as bass
import concourse.mybir as mybir
import concourse.tile as tile
from concourse import bacc
from concourse.bass_utils import run_bass_kernel_spmd

BATCH = 32768
N_MF = 64
CONSEQ = N_MF * N_MF  # 4096
N_CORES = 8
SHARD = BATCH // N_CORES  # 4096
P = 128  # partitions
HEAD_OPT = True  # first input chunk on sync ring + finer tile-0 splits


def build_kernel(tc: tile.TileContext, out_ap: bass.AP, x_ap: bass.AP, shard: int = SHARD):
    """Per-core kernel body. x_ap: [shard, 2, 64] f32, out_ap: [shard, 4096] f32."""
    nc = tc.nc
    n_tiles = shard // P
    with ExitStack() as ctx:
        in_pool = ctx.enter_context(tc.tile_pool(name="xin", bufs=1))
        out_pool = ctx.enter_context(tc.tile_pool(name="out", bufs=6))

        # Chunked load of the shard: [shard, 2, 64] -> SBUF [128, n_tiles*128]
        # xt[p, t*128 + i*64 + m] = x[t*128 + p, i, m]
        # First chunk is small so compute can start ASAP; input DMAs ride the
        # ACT HWDGE ring (nc.scalar) to stay out of the output ring's FIFO.
        xt = in_pool.tile([P, n_tiles * 2 * N_MF], mybir.dt.float32)
        xt3 = xt[:].rearrange("p (t k) -> p t k", t=n_tiles)
        xd3 = x_ap.rearrange("(t p) i m -> p t (i m)", p=P)
        chunks = [1, 1, 2, 4] + [4] * ((n_tiles - 8) // 4) if n_tiles >= 8 else [1] * n_tiles
        t0 = 0
        for ci, c in enumerate(chunks):
            eng = nc.sync if (HEAD_OPT and ci == 0) else nc.scalar
            eng.dma_start(xt3[:, t0 : t0 + c, :], xd3[:, t0 : t0 + c, :])
            t0 += c
        assert t0 == n_tiles

        # j1 blocks [0, D_J1) go to DVE as one broadcast tensor_tensor; blocks
        # [D_J1, 64) go to ACT as per-block activation(Copy, scale=a[:, j1]).
        # The DVE runs 4.42-5.30us per full tile depending on (environmental)
        # clock mode; offloading ~1/5 to ACT keeps compute under the DMA rate
        # in the slow mode. Tile 0 stays DVE-only, quartered for an early
        # first output DMA.
        D_J1 = 54
        for t in range(n_tiles):
            a = xt[:, t * 128 : t * 128 + N_MF]           # x[rows, 0, :]
            b = xt[:, t * 128 + N_MF : t * 128 + 2 * N_MF]  # x[rows, 1, :]
            ot = out_pool.tile([P, CONSEQ], mybir.dt.float32)
            if t == 0:
                t0_splits = [8, 8, 16, 16, 16] if HEAD_OPT else [16, 16, 16, 16]
                j0 = 0
                for jw in t0_splits:
                    nc.vector.tensor_mul(
                        ot[:, j0 * N_MF : (j0 + jw) * N_MF].rearrange(
                            "p (a b) -> p a b", a=jw
                        ),
                        a[:, j0 : j0 + jw].unsqueeze(2).to_broadcast([P, jw, N_MF]),
                        b.unsqueeze(1).to_broadcast([P, jw, N_MF]),
                    )
                    nc.sync.dma_start(
                        out_ap[t * P : (t + 1) * P, j0 * N_MF : (j0 + jw) * N_MF],
                        ot[:, j0 * N_MF : (j0 + jw) * N_MF],
                    )
                    j0 += jw
                continue
            nc.vector.tensor_mul(
                ot[:, : D_J1 * N_MF].rearrange("p (a b) -> p a b", a=D_J1),
                a[:, :D_J1].unsqueeze(2).to_broadcast([P, D_J1, N_MF]),
                b.unsqueeze(1).to_broadcast([P, D_J1, N_MF]),
            )
            for j1 in range(D_J1, N_MF):
                nc.scalar.activation(
                    ot[:, j1 * N_MF : (j1 + 1) * N_MF],
                    b,
                    mybir.ActivationFunctionType.Copy,
                    scale=a[:, j1 : j1 + 1],
                )
            nc.sync.dma_start(out_ap[t * P : (t + 1) * P, :], ot[:])


def _ensure_trace_support():
    """Install the NTFF profile hook that the slim agent image omits.

    run_bass_kernel_spmd(trace=True) under axon imports
    antenv.axon_hooks.get_axon_ntff_profile_hook; the container's antenv
    stub lacks that module. Recreate it in sys.modules, backed by the
    ctypes hook factory in trn_agent_boot.trn_boot.
    """
    import sys
    import types

    if "antenv.axon_hooks" in sys.modules:
        return
    try:
        from trn_agent_boot.trn_boot import _ntff_profile_via_ctypes

        hook = _ntff_profile_via_ctypes("/opt/axon/libaxon_pjrt.so")
    except Exception:
        hook = None
    mod = types.ModuleType("antenv.axon_hooks")
    _state = {"hook": hook}
    mod.get_axon_ntff_profile_hook = lambda: _state["hook"]
    mod.set_axon_ntff_profile_hook = lambda h: _state.__setitem__("hook", h)
    sys.modules["antenv.axon_hooks"] = mod
    import antenv

    antenv.axon_hooks = mod


_CACHED = {}


def _build(shard: int = SHARD):
    key = shard
    if key in _CACHED:
        return _CACHED[key]
    nc = bacc.Bacc(
        "TRN2",
        target_bir_lowering=False,
        debug=False,
        enable_asserts=False,
        num_devices=N_CORES,
    )
    x_t = nc.dram_tensor("x", [shard, 2, N_MF], mybir.dt.float32, kind="ExternalInput")
    out_t = nc.dram_tensor("out", [shard, CONSEQ], mybir.dt.float32, kind="ExternalOutput")
    with tile.TileContext(nc) as tc:
        build_kernel(tc, out_t.ap(), x_t.ap(), shard)
    nc.compile()
    _CACHED[key] = nc
    return nc


def _run(x: np.ndarray, trace: bool = False):
    """Run on 8 cores. Returns (out [32768,4096] f32, BassKernelResults)."""
    x = np.ascontiguousarray(np.asarray(x, dtype=np.float32))
    assert x.shape == (BATCH, 2, N_MF), x.shape
    if trace:
        _ensure_trace_support()
    nc = _build()
    in_maps = [{"x": x[c * SHARD : (c + 1) * SHARD]} for c in range(N_CORES)]
    res = run_bass_kernel_spmd(nc, in_maps, core_ids=list(range(N_CORES)), trace=trace)
    out = np.concatenate([res.results[c]["out"] for c in range(N_CORES)], axis=0)
    return out, res


def kernel(**inputs: np.ndarray) -> np.ndarray:
    out, _ = _run(inputs["x"], trace=bool(int(os.environ.get("KERNEL_TRACE", "0"))))
    return out


# revision 24
# speedup vs baseline: 1.0023x; 1.0023x over previous
"""Trainium2 Bass kernel for nn_ANFISRuleStrengthLayer.

Math (N_INPUTS=2, N_MF=64): out[b, j1*64 + j2] = x[b, 0, j1] * x[b, 1, j2]
i.e. a per-sample outer product of two 64-vectors.
Input  x:   (32768, 2, 64) f32
Output out: (32768, 4096)  f32   (512 MiB -> heavily DMA-write bound)

Sharding: pure data parallel over batch across 8 NeuronCores
(4096 rows per core, no cross-core communication).
"""

import os
from contextlib import ExitStack

import numpy as np

import concourse.bass as bass
import concourse.mybir as mybir
import concourse.tile as tile
from concourse import bacc
from concourse.bass_utils import run_bass_kernel_spmd

BATCH = 32768
N_MF = 64
CONSEQ = N_MF * N_MF  # 4096
N_CORES = 8
SHARD = BATCH // N_CORES  # 4096
P = 128  # partitions
HEAD_OPT = True  # first input chunk on sync ring + finer tile-0 splits


def build_kernel(tc: tile.TileContext, out_ap: bass.AP, x_ap: bass.AP, shard: int = SHARD):
    """Per-core kernel body. x_ap: [shard, 2, 64] f32, out_ap: [shard, 4096] f32."""
    nc = tc.nc
    n_tiles = shard // P
    with ExitStack() as ctx:
        in_pool = ctx.enter_context(tc.tile_pool(name="xin", bufs=1))
        out_pool = ctx.enter_context(tc.tile_pool(name="out", bufs=6))

        # Chunked load of the shard: [shard, 2, 64] -> SBUF [128, n_tiles*128]
        # xt[p, t*128 + i*64 + m] = x[t*128 + p, i, m]
        # First chunk is small so compute can start ASAP; input DMAs ride the
        # ACT HWDGE ring (nc.scalar) to stay out of the output ring's FIFO.
        xt = in_pool.tile([P, n_tiles * 2 * N_MF], mybir.dt.float32)
        xt3 = xt[:].rearrange("p (t k) -> p t k", t=n_tiles)
        xd3 = x_ap.rearrange("(t p) i m -> p t (i m)", p=P)
        chunks = [1, 1, 2, 4] + [4] * ((n_tiles - 8) // 4) if n_tiles >= 8 else [1] * n_tiles
        t0 = 0
        for ci, c in enumerate(chunks):
            eng = nc.sync if (HEAD_OPT and ci <= 2) else nc.scalar
            eng.dma_start(xt3[:, t0 : t0 + c, :], xd3[:, t0 : t0 + c, :])
            t0 += c
        assert t0 == n_tiles

        # j1 blocks [0, D_J1) go to DVE as one broadcast tensor_tensor; blocks
        # [D_J1, 64) go to ACT as per-block activation(Copy, scale=a[:, j1]).
        # The DVE runs 4.42-5.30us per full tile depending on (environmental)
        # clock mode; offloading ~1/5 to ACT keeps compute under the DMA rate
        # in the slow mode. Tile 0 stays DVE-only, quartered for an early
        # first output DMA.
        D_J1 = 54
        for t in range(n_tiles):
            a = xt[:, t * 128 : t * 128 + N_MF]           # x[rows, 0, :]
            b = xt[:, t * 128 + N_MF : t * 128 + 2 * N_MF]  # x[rows, 1, :]
            ot = out_pool.tile([P, CONSEQ], mybir.dt.float32)
            if t == 0:
                t0_splits = [8, 8, 16, 16, 16] if HEAD_OPT else [16, 16, 16, 16]
                j0 = 0
                for jw in t0_splits:
                    nc.vector.tensor_mul(
                        ot[:, j0 * N_MF : (j0 + jw) * N_MF].rearrange(
                            "p (a b) -> p a b", a=jw
                        ),
                        a[:, j0 : j0 + jw].unsqueeze(2).to_broadcast([P, jw, N_MF]),
                        b.unsqueeze(1).to_broadcast([P, jw, N_MF]),
                    )
                    nc.sync.dma_start(
                        out_ap[t * P : (t + 1) * P, j0 * N_MF : (j0 + jw) * N_MF],
                        ot[:, j0 * N_MF : (j0 + jw) * N_MF],
                    )
                    j0 += jw
                continue
            nc.vector.tensor_mul(
                ot[:, : D_J1 * N_MF].rearrange("p (a b) -> p a b", a=D_J1),
                a[:, :D_J1].unsqueeze(2).to_broadcast([P, D_J1, N_MF]),
                b.unsqueeze(1).to_broadcast([P, D_J1, N_MF]),
            )
            for j1 in range(D_J1, N_MF):
                nc.scalar.activation(
                    ot[:, j1 * N_MF : (j1 + 1) * N_MF],
                    b,
                    mybir.ActivationFunctionType.Copy,
                    scale=a[:, j1 : j1 + 1],
                )
            nc.sync.dma_start(out_ap[t * P : (t + 1) * P, :], ot[:])


def _ensure_trace_support():
    """Install the NTFF profile hook that the slim agent image omits.

    run_bass_kernel_spmd(trace=True) under axon imports
    antenv.axon_hooks.get_axon_ntff_profile_hook; the container's antenv
    stub lacks that module. Recreate it in sys.modules, backed by the
    ctypes hook factory in trn_agent_boot.trn_boot.
    """
    import sys
    import types

    if "antenv.axon_hooks" in sys.modules:
        return
    try:
        from trn_agent_boot.trn_boot import _ntff_profile_via_ctypes

        hook = _ntff_profile_via_ctypes("/opt/axon/libaxon_pjrt.so")
    except Exception:
        hook = None
    mod = types.ModuleType("antenv.axon_hooks")
    _state = {"hook": hook}
    mod.get_axon_ntff_profile_hook = lambda: _state["hook"]
    mod.set_axon_ntff_profile_hook = lambda h: _state.__setitem__("hook", h)
    sys.modules["antenv.axon_hooks"] = mod
    import antenv

    antenv.axon_hooks = mod


_CACHED = {}


def _build(shard: int = SHARD):
    key = shard
    if key in _CACHED:
        return _CACHED[key]
    nc = bacc.Bacc(
        "TRN2",
        target_bir_lowering=False,
        debug=False,
        enable_asserts=False,
        num_devices=N_CORES,
    )
    x_t = nc.dram_tensor("x", [shard, 2, N_MF], mybir.dt.float32, kind="ExternalInput")
    out_t = nc.dram_tensor("out", [shard, CONSEQ], mybir.dt.float32, kind="ExternalOutput")
    with tile.TileContext(nc) as tc:
        build_kernel(tc, out_t.ap(), x_t.ap(), shard)
    nc.compile()
    _CACHED[key] = nc
    return nc


def _run(x: np.ndarray, trace: bool = False):
    """Run on 8 cores. Returns (out [32768,4096] f32, BassKernelResults)."""
    x = np.ascontiguousarray(np.asarray(x, dtype=np.float32))
    assert x.shape == (BATCH, 2, N_MF), x.shape
    if trace:
        _ensure_trace_support()
    nc = _build()
    in_maps = [{"x": x[c * SHARD : (c + 1) * SHARD]} for c in range(N_CORES)]
    res = run_bass_kernel_spmd(nc, in_maps, core_ids=list(range(N_CORES)), trace=trace)
    out = np.concatenate([res.results[c]["out"] for c in range(N_CORES)], axis=0)
    return out, res


def kernel(**inputs: np.ndarray) -> np.ndarray:
    out, _ = _run(inputs["x"], trace=bool(int(os.environ.get("KERNEL_TRACE", "0"))))
    return out
